# revision 3
# baseline (speedup 1.0000x reference)
"""Trainium2 Bass kernel for nn_ComplexShiftedWindowAttnBlock.

Strategy: data-parallel over tokens across 8 NeuronCores. The device runs a
single compiled Bass/Tile program: a complex GEMM  O = W @ X  with
K=256 (contraction), Cout=1024 (padded), T=1568 tokens per core, realized as
4 real fp32r matmuls accumulated in PSUM. All four linear layers of each
branch (qkv / proj / fc1 / fc2-in-4-K-chunks) — ~94% of the FLOPs — run on
device; window partitioning, layernorm, softmax, relative-position bias,
shift masks and residuals are glue done host-side in float32 numpy,
mirroring the reference exactly.
"""

import numpy as np

import concourse.bacc as bacc
import concourse.bass as bass
import concourse.mybir as mybir
import concourse.tile as tile
from concourse.bass_utils import run_bass_kernel_spmd

WS = 7
SHIFT = 3
NUM_HEADS = 8
C = 256
EPS = 1e-5
NCORES = 8
T = 1568            # tokens per core (12544 / 8)
TT = 392            # free-dim tile (4 per core)
COUT = 1024
F32 = mybir.dt.float32
F32R = mybir.dt.float32r

_NC = None
HW_NS = 0.0         # accumulated device exec time (ns) across calls, best effort


def _build_nc():
    nc = bacc.Bacc("TRN2", target_bir_lowering=False, debug=False, num_devices=NCORES)
    xr = nc.dram_tensor("xr", [128, 2 * T], F32R, kind="ExternalInput").ap()
    xi = nc.dram_tensor("xi", [128, 2 * T], F32R, kind="ExternalInput").ap()
    wrt = nc.dram_tensor("wrt", [128, 2 * COUT], F32R, kind="ExternalInput").ap()
    wit = nc.dram_tensor("wit", [128, 2 * COUT], F32R, kind="ExternalInput").ap()
    wnt = nc.dram_tensor("wnt", [128, 2 * COUT], F32R, kind="ExternalInput").ap()
    o_re = nc.dram_tensor("o_re", [COUT, T], F32, kind="ExternalOutput").ap()
    o_im = nc.dram_tensor("o_im", [COUT, T], F32, kind="ExternalOutput").ap()

    with tile.TileContext(nc) as tc:
        with (
            tc.tile_pool(name="xp", bufs=1) as xp,
            tc.tile_pool(name="wp", bufs=1) as wp,
            tc.tile_pool(name="op", bufs=6) as op,
            tc.tile_pool(name="pp", bufs=3, space="PSUM") as pp,
        ):
            xr_sb = xp.tile([128, 2 * T], F32R, tag="xr")
            nc.sync.dma_start(xr_sb[:], xr)
            xi_sb = xp.tile([128, 2 * T], F32R, tag="xi")
            nc.sync.dma_start(xi_sb[:], xi)
            wr_sb = wp.tile([128, 2 * COUT], F32R, tag="wr")
            nc.sync.dma_start(wr_sb[:], wrt)
            wi_sb = wp.tile([128, 2 * COUT], F32R, tag="wi")
            nc.sync.dma_start(wi_sb[:], wit)
            wn_sb = wp.tile([128, 2 * COUT], F32R, tag="wn")
            nc.sync.dma_start(wn_sb[:], wnt)

            for co in range(COUT // 128):
                for ti in range(T // TT):
                    pre = pp.tile([128, TT], F32, tag="pre")
                    pim = pp.tile([128, TT], F32, tag="pim")
                    for ci in range(2):
                        ws = slice(ci * COUT + co * 128, ci * COUT + (co + 1) * 128)
                        xs = slice(ci * T + ti * TT, ci * T + (ti + 1) * TT)
                        w_r = wr_sb[:, ws]
                        w_i = wi_sb[:, ws]
                        w_n = wn_sb[:, ws]
                        x_r = xr_sb[:, xs]
                        x_i = xi_sb[:, xs]
                        # o_re = Wr@Xr + (-Wi)@Xi ; o_im = Wr@Xi + Wi@Xr
                        nc.tensor.matmul(pre[:], w_r, x_r, start=(ci == 0), stop=False)
                        nc.tensor.matmul(pre[:], w_n, x_i, start=False, stop=(ci == 1))
                        nc.tensor.matmul(pim[:], w_r, x_i, start=(ci == 0), stop=False)
                        nc.tensor.matmul(pim[:], w_i, x_r, start=False, stop=(ci == 1))
                    ore = op.tile([128, TT], F32, tag="ore")
                    nc.vector.tensor_copy(ore[:], pre[:])
                    oim = op.tile([128, TT], F32, tag="oim")
                    nc.scalar.copy(oim[:], pim[:])
                    rs = slice(co * 128, (co + 1) * 128)
                    cs = slice(ti * TT, (ti + 1) * TT)
                    nc.sync.dma_start(o_re[rs, cs], ore[:])
                    nc.sync.dma_start(o_im[rs, cs], oim[:])
    nc.compile()
    return nc


def _sb(a):
    # (256, N) -> (128, 2N) with column blocks [ci*N + j]
    n = a.shape[1]
    return np.ascontiguousarray(
        a.reshape(2, 128, n).transpose(1, 0, 2).reshape(128, 2 * n), dtype=np.float32
    )


def _dev_cgemm(X, Wc):
    """X: (12544, 256) complex64, Wc: (Cout<=1024, 256) complex64.
    Returns X @ Wc.T as (12544, Cout) complex64, computed on 8 NeuronCores."""
    global _NC, HW_NS
    if _NC is None:
        _NC = _build_nc()
    cout = Wc.shape[0]
    WT = np.zeros((256, COUT), np.complex64)
    WT[:, :cout] = Wc.T
    wr = _sb(WT.real)
    wi = _sb(WT.imag)
    wn = _sb(-WT.imag)
    Xs = X.reshape(NCORES, T, 256)
    in_maps = []
    for c in range(NCORES):
        xc = Xs[c].T  # (256, T)
        in_maps.append(
            {
                "xr": _sb(np.ascontiguousarray(xc.real)),
                "xi": _sb(np.ascontiguousarray(xc.imag)),
                "wrt": wr,
                "wit": wi,
                "wnt": wn,
            }
        )
    res = run_bass_kernel_spmd(_NC, in_maps, core_ids=list(range(NCORES)))
    if getattr(res, "exec_time_ns", None):
        HW_NS += float(res.exec_time_ns)
    outs = []
    for c in range(NCORES):
        r = res.results[c]
        o = (r["o_re"][:cout] + 1j * r["o_im"][:cout]).astype(np.complex64)
        outs.append(o.T)  # (T, cout)
    return np.concatenate(outs, axis=0)


# ---------------- host-side glue (numpy float32, mirrors reference) -------


def _rel_pos_index(ws):
    coords = np.stack(np.meshgrid(np.arange(ws), np.arange(ws), indexing="ij"))
    cf = coords.reshape(2, -1)
    rel = (cf[:, :, None] - cf[:, None, :]).transpose(1, 2, 0)
    rel[:, :, 0] += ws - 1
    rel[:, :, 1] += ws - 1
    rel[:, :, 0] *= 2 * ws - 1
    return rel.sum(-1)


RPI = _rel_pos_index(WS)


def _shift_mask(h, w, ws, ss):
    img = np.zeros((h, w), np.float32)
    sls = (slice(0, -ws), slice(-ws, -ss), slice(-ss, None))
    cnt = 0
    for i in sls:
        for j in sls:
            img[i, j] = cnt
            cnt += 1
    mw = img.reshape(h // ws, ws, w // ws, ws).transpose(0, 2, 1, 3).reshape(-1, ws * ws)
    am = mw[:, None, :] - mw[:, :, None]
    return np.where(am != 0, np.float32(-100.0), np.float32(0.0))


def window_partition(x, ws):
    b, h, w, c = x.shape
    x = x.reshape(b, h // ws, ws, w // ws, ws, c)
    return x.transpose(0, 1, 3, 2, 4, 5).reshape(-1, ws, ws, c)


def window_reverse(wins, ws, h, w):
    c = wins.shape[-1]
    b = wins.shape[0] // ((h // ws) * (w // ws))
    x = wins.reshape(b, h // ws, w // ws, ws, ws, c)
    return x.transpose(0, 1, 3, 2, 4, 5).reshape(b, h, w, c)


def _ln(r):
    mu = r.mean(-1, keepdims=True, dtype=np.float32)
    v = ((r - mu) ** 2).mean(-1, keepdims=True, dtype=np.float32)
    return (r - mu) / np.sqrt(v + np.float32(EPS))


def cln(x, g, b):
    return (_ln(x.real) + 1j * _ln(x.imag)).astype(np.complex64) * g + b


def _softmax(a):
    a = a - a.max(-1, keepdims=True)
    e = np.exp(a, dtype=np.float32)
    return e / e.sum(-1, keepdims=True, dtype=np.float32)


def csoftmax(a):
    return (_softmax(a.real) + 1j * _softmax(a.imag)).astype(np.complex64)


def _gelu(x):
    c0 = np.float32(0.7978845608028654)  # sqrt(2/pi)
    c1 = np.float32(0.044715)
    return np.float32(0.5) * x * (np.float32(1.0) + np.tanh(c0 * (x + c1 * x * x * x)))


def cgelu(x):
    return (_gelu(x.real) + 1j * _gelu(x.imag)).astype(np.complex64)


def window_attention(xw, p, mask):
    B_, N, c = xw.shape
    H = NUM_HEADS
    d = c // H
    scale = np.float32(d ** (-0.5))
    qkv = _dev_cgemm(xw.reshape(-1, c), p["qkv_w"]) + p["qkv_b"]
    qkv = qkv.reshape(B_, N, 3, H, d).transpose(2, 0, 3, 1, 4)
    q, k, v = qkv[0] * scale, qkv[1], qkv[2]
    attn = np.matmul(q, k.transpose(0, 1, 3, 2))  # (B_, H, N, N) complex
    rpb = p["rpb"][RPI.reshape(-1)].reshape(N, N, H).transpose(2, 0, 1)
    attn = attn + rpb[None]
    if mask is not None:
        nW = mask.shape[0]
        attn = attn.reshape(B_ // nW, nW, H, N, N) + mask[None, :, None]
        attn = attn.reshape(B_, H, N, N)
    attn = csoftmax(attn)
    out = np.matmul(attn, v).transpose(0, 2, 1, 3).reshape(B_, N, c)
    return _dev_cgemm(out.reshape(-1, c), p["proj_w"]).reshape(B_, N, c) + p["proj_b"]


def mlp(x, p):
    n = x.shape[0]
    h1 = _dev_cgemm(x, p["fc1_w"]) + p["fc1_b"]
    h1 = cgelu(h1)
    out = np.zeros((n, C), np.complex64)
    for kchunk in range(4):
        cols = slice(kchunk * 256, (kchunk + 1) * 256)
        out += _dev_cgemm(
            np.ascontiguousarray(h1[:, cols]),
            np.ascontiguousarray(p["fc2_w"][:, cols]),
        )
    return out + p["fc2_b"]


def _forward(x, params):
    b, h, w, c = x.shape
    x = x.reshape(b, -1, c)
    shortcut = x
    xn = cln(x, params["w_norm1_g"], params["w_norm1_b"]).reshape(b, h, w, c)
    xw = window_partition(xn, WS).reshape(-1, WS * WS, c)
    aw = window_attention(xw, params["w_attn"], None).reshape(-1, WS, WS, c)
    x = shortcut + window_reverse(aw, WS, h, w).reshape(b, -1, c)
    xf = x.reshape(-1, c)
    x = x + mlp(
        cln(xf.reshape(b, -1, c), params["w_norm2_g"], params["w_norm2_b"]).reshape(-1, c),
        params["w_mlp"],
    ).reshape(b, -1, c)
    # shifted branch
    xn = cln(x, params["sw_norm1_g"], params["sw_norm1_b"]).reshape(b, h, w, c)
    mask = _shift_mask(h, w, WS, SHIFT)
    xs = np.roll(xn, (-SHIFT, -SHIFT), axis=(1, 2))
    xw = window_partition(xs, WS).reshape(-1, WS * WS, c)
    aw = window_attention(xw, params["sw_attn"], mask).reshape(-1, WS, WS, c)
    xs = window_reverse(aw, WS, h, w)
    xs = np.roll(xs, (SHIFT, SHIFT), axis=(1, 2)).reshape(b, -1, c)
    xs = xn.reshape(b, -1, c) + xs
    x = xs + mlp(
        cln(xs, params["sw_norm2_g"], params["sw_norm2_b"]).reshape(-1, c),
        params["sw_mlp"],
    ).reshape(b, -1, c)
    return x.reshape(b, h, w, c)


def _np_tree(p):
    if isinstance(p, dict):
        return {k: _np_tree(v) for k, v in p.items()}
    return np.asarray(p)


def kernel(x_re, x_im, params):
    global HW_NS
    HW_NS = 0.0
    params = _np_tree(params)
    x = (np.asarray(x_re) + 1j * np.asarray(x_im)).astype(np.complex64)
    return _forward(x, params)
